# revision 2
# baseline (speedup 1.0000x reference)
"""AxialAttention3D Trainium2 kernel (v2).

Reference computes, for each of 3 weight branches (d/h/w), full global
multi-head attention over the flattened 16^3 = 4096 spatial positions of
x (1, 128, 16, 16, 16), 8 heads x dim_head 16, then
    out = gamma * (out_d + out_h + out_w) + x.

Sharding: core c computes head c of all 3 branches (3 units/core); the
host sums the 8 partial projected outputs and adds the residual x.

v2 bottleneck analysis: the baseline was ScalarE-bound — softmax exp of
3 x 4096^2 scores at 1 elem/lane/cycle (445us busy).  PSUM->SBUF can
only cross through ACT or DVE, so v2 splits the exp stream across BOTH:
  - ACT items: native exp activation (scale=0.25) -> bf16.
  - DVE items: Schraudolph bit-trick — one tensor_scalar op computes
    round(A*s + B) into an int16 view of the bf16 P tile; the int16 bit
    pattern IS bf16 exp(0.25 s) within +-3.3% (verified on HW: RNE
    convert).  Softmax's num/denom ratio cancels the systematic part.
Bias folding: q/k biases fold into an extra contraction row (k side
carries r.x = bq.W_k x from an extra lk column; the shared +1 offset
from the copy's per-partition add is softmax-invariant), so scores
matmuls are K=17 row-tiled at 4 band offsets.  attn@V matmuls are
col-tiled per unit (tile_position=(0,32u)) and u-interleaved so 3 col
bands run concurrently.  V bias + out-proj bias fold into beff carried
by wo row 96 against a constant ones row of `scaled`.
"""

import math

import numpy as np


def _bf16np():
    import ml_dtypes

    return ml_dtypes.bfloat16


HEADS = 8
DH = 16
C = 128
NCORES = 8

A_SCH = 46.16624130844683  # 0.25 * 128 / ln 2
B_SCH = 16249.25

_FULL = dict(MT=32, CHUNK=512, NCH=8, GRP=3, ACT_FRAC=0.56, LAG=2, EPI_DELAY=3)
_CACHE = {}


def _patch_tile_drain():
    """walrus in this env rejects >1 sync wait on one instruction; split the
    Tile kernel-tail drain's aggregated waits into one drain per wait."""
    import concourse.mybir as mybir
    from concourse.tile import TileContext, ScopedClock

    if getattr(TileContext, "_drain_split_patched", False):
        return

    def _drain_and_barrier_split(self, tick_clock, wait_clock):
        probe = self.nc.sync.drain()
        wait_clock.add_sem_waits(
            probe.ins, ScopedClock({None: tick_clock.global_clock})
        )
        si = probe.ins.sync_info
        waits = list(si.on_wait) if si is not None else []
        if len(waits) > 1:
            si.on_wait = [waits[0]]
            for w in waits[1:]:
                d = self.nc.sync.drain()
                d.ins.sync_info = mybir.SyncInfo(on_wait=[w], on_update=[])
        self.nc.all_engine_barrier()
        assert self.sems is not None
        popped = self.nc._tile_sem_poison_stack.pop()
        assert popped is self._sem_poison
        self.nc.clear_and_free_semaphores(list(self.sems.allocated().values()))
        self.nc.all_engine_barrier()

    TileContext._drain_and_barrier = _drain_and_barrier_split
    TileContext._drain_split_patched = True


def _split_multi_waits(nc):
    """walrus in this env allows at most ONE sync wait per instruction.
    Hoist extra waits onto same-engine NoOps inserted just before."""
    import concourse.mybir as mybir

    for f in nc.m.functions:
        for bb in f.blocks:
            new = []
            changed = False
            for inst in bb.instructions:
                si = inst.sync_info
                if si is not None and si.on_wait and len(si.on_wait) > 1:
                    waits = list(si.on_wait)
                    for j, w in enumerate(waits[:-1]):
                        nop = mybir.InstNoOp(
                            name=f"{inst.name}-w{j}",
                            engine=inst.engine,
                            sync_info=mybir.SyncInfo(on_wait=[w], on_update=[]),
                            bass_nofuse=True,
                        )
                        new.append(nop)
                    si.on_wait = [waits[-1]]
                    changed = True
                new.append(inst)
            if changed:
                bb.instructions = new


def build_nc(cfg=_FULL, split_waits=True):
    import concourse.bass as bass
    import concourse.mybir as mybir
    from concourse import tile

    _patch_tile_drain()

    f32 = mybir.dt.float32
    f32r = mybir.dt.float32r
    bf16 = mybir.dt.bfloat16
    i16 = mybir.dt.int16
    Exp = mybir.ActivationFunctionType.Exp
    Ident = mybir.ActivationFunctionType.Identity
    Mult = mybir.AluOpType.mult
    Add = mybir.AluOpType.add

    MT, CHUNK, NCH, GRP = cfg["MT"], cfg["CHUNK"], cfg["NCH"], cfg["GRP"]
    LAG, EPI_DELAY = cfg["LAG"], cfg["EPI_DELAY"]
    N = MT * 128
    assert N == CHUNK * NCH
    TPC = CHUNK // 128  # m-tiles per chunk (4)

    groups = []
    t0 = 0
    while t0 < MT:
        groups.append(list(range(t0, min(t0 + GRP, MT))))
        t0 += GRP
    NGRP = len(groups)
    per_chunk = 3 * NGRP

    nc = bass.Bass("TRN2", target_bir_lowering=False, debug=False)

    x_d = nc.declare_dram_parameter("x", [C, N], bf16, isOutput=False)
    lq_d = [
        nc.declare_dram_parameter(f"lq{u}", [C, 128], bf16, isOutput=False)
        for u in range(3)
    ]
    lk_d = [
        nc.declare_dram_parameter(f"lk{u}", [C, 128], bf16, isOutput=False)
        for u in range(3)
    ]
    addv_d = nc.declare_dram_parameter("addv", [C, 1], f32, isOutput=False)
    wv_d = nc.declare_dram_parameter("wv3", [C, 52], bf16, isOutput=False)
    wo_d = nc.declare_dram_parameter("wo", [C, 128], f32r, isOutput=False)
    onesr_d = nc.declare_dram_parameter("onesr", [C, CHUNK], f32r, isOutput=False)
    y_d = nc.declare_dram_parameter("y", [C, N], f32, isOutput=True)

    # exp engine schedule: True -> ACT, False -> DVE (Schraudolph)
    n_items = NCH * NGRP * 3
    act_frac = cfg["ACT_FRAC"]
    eng_act = []
    accf = 0.0
    for _ in range(n_items):
        accf += act_frac
        if accf >= 1.0:
            eng_act.append(True)
            accf -= 1.0
        else:
            eng_act.append(False)

    with tile.TileContext(nc) as tc:
        with (
            tc.tile_pool(name="persist", bufs=1) as pp,
            tc.tile_pool(name="pt", bufs=4) as ptp,
            tc.tile_pool(name="osb", bufs=2) as osbp,
            tc.tile_pool(name="big", bufs=2, space="PSUM") as bigp,
            tc.tile_pool(name="accp", bufs=1, space="PSUM") as accp,
            tc.tile_pool(name="projp", bufs=1, space="PSUM") as projp,
        ):
            # ---- persistent SBUF tensors ----
            x_sb = pp.tile([C, N], bf16, name="x_sb", tag="x")
            for cidx in range(NCH):
                nc.sync.dma_start(
                    x_sb[:, cidx * CHUNK : (cidx + 1) * CHUNK],
                    x_d[:, cidx * CHUNK : (cidx + 1) * CHUNK],
                )
            lq = [pp.tile([C, 128], bf16, name=f"lq{u}_sb", tag=f"lq{u}") for u in range(3)]
            lk = [pp.tile([C, 128], bf16, name=f"lk{u}_sb", tag=f"lk{u}") for u in range(3)]
            addv = pp.tile([C, 1], f32, name="addv_sb", tag="addv")
            wv = pp.tile([C, 52], bf16, name="wv_sb", tag="wv")
            wo = pp.tile([C, 128], f32r, name="wo_sb", tag="wo")
            for u in range(3):
                nc.sync.dma_start(lq[u][:], lq_d[u][:])
                nc.sync.dma_start(lk[u][:], lk_d[u][:])
            nc.sync.dma_start(addv[:], addv_d[:])
            nc.sync.dma_start(wv[:], wv_d[:])
            nc.sync.dma_start(wo[:], wo_d[:])

            # qk[u]: per chunk c, [c*1024, +512) = q, [c*1024+512, +1024) = k
            qk = [pp.tile([C, 2 * N], bf16, name=f"qk{u}_sb", tag=f"qk{u}") for u in range(3)]
            vT = pp.tile([C, MT * 51], bf16, name="vT_sb", tag="vT")
            dstage = pp.tile([C, CHUNK], f32, name="dstage_sb", tag="dstage")
            denb = pp.tile([C, 16], f32, name="denb_sb", tag="denb")
            recb = pp.tile([C, 16], f32, name="recb_sb", tag="recb")
            normsb = pp.tile([C, CHUNK], f32, name="normsb_sb", tag="normsb")
            scaled = pp.tile([C, CHUNK], f32r, name="scaled_sb", tag="scaled")
            nc.sync.dma_start(scaled[:], onesr_d[:])

            qk_copy_flip = [True]  # alternate ACT/DVE for qk copies

            def emit_qk(u, cidx):
                cs, ce = cidx * CHUNK, (cidx + 1) * CHUNK
                ps = bigp.tile([C, 2 * CHUNK], f32, name="qkps", tag="scores")
                nc.tensor.matmul(
                    ps[:, 0:CHUNK], lhsT=lq[u][:], rhs=x_sb[:, cs:ce],
                    start=True, stop=True,
                )
                nc.tensor.matmul(
                    ps[:, CHUNK : 2 * CHUNK], lhsT=lk[u][:], rhs=x_sb[:, cs:ce],
                    start=True, stop=True,
                )
                dst = qk[u][:, cidx * 2 * CHUNK : (cidx + 1) * 2 * CHUNK]
                if qk_copy_flip[0]:
                    nc.scalar.activation(dst, ps[:], Ident, bias=addv[:], scale=1.0)
                else:
                    nc.vector.tensor_scalar_add(dst, ps[:], addv[:])
                qk_copy_flip[0] = not qk_copy_flip[0]

            def emit_vt(B):
                # batch of 4 m-tiles -> one copy + ones memset
                ts = [4 * B + i for i in range(4)]
                ps = bigp.tile([C, 51 * 4], f32, name="vps", tag="scores")
                for i, t in enumerate(ts):
                    nc.tensor.matmul(
                        ps[:, i * 51 : (i + 1) * 51],
                        lhsT=x_sb[:, t * 128 : (t + 1) * 128],
                        rhs=wv[:, 0:51],
                        start=True, stop=True,
                    )
                dst = vT[:, ts[0] * 51 : (ts[-1] + 1) * 51]
                nc.vector.tensor_copy(dst, ps[:])
                ones_ap = dst.rearrange("p (t u d) -> p (t u) d", u=3, d=17)[:, :, 16]
                nc.vector.memset(ones_ap, 1.0)

            # ---- item schedule ----
            items = [
                (c, g, u) for c in range(NCH) for g in range(NGRP) for u in range(3)
            ]

            # drip deadlines for remaining qk / vt emissions
            drip = []
            for ck in range(1, NCH):
                g_need = math.ceil((4 * ck - 2) / 3)
                dl = min(ck * per_chunk, 3 * g_need)
                for u in range(3):
                    drip.append((max(0, dl + u - 2), "qk", (u, ck)))
            for B in range(1, MT // 4):
                dl = 3 * ((4 * B) // GRP)
                drip.append((max(0, dl - 2), "vt", B))
            drip.sort(key=lambda z: z[0])

            pt_of_item = {}
            acc_of_chunk = {}
            pending_proj = []

            def emit_scores(idx):
                c, g, u = items[idx]
                tlist = groups[g]
                cs, ce = c * CHUNK, (c + 1) * CHUNK
                sc = bigp.tile(
                    [C, CHUNK * len(tlist)], f32, name="sc_ps", tag="scores"
                )
                qs = c * 2 * CHUNK
                for i, t in enumerate(tlist):
                    r = t % 4
                    ck, ko = t // TPC, (t % TPC) * 128
                    kbase = ck * 2 * CHUNK + CHUNK + ko
                    nc.tensor.matmul(
                        sc[:, i * CHUNK : (i + 1) * CHUNK],
                        lhsT=qk[u][32 * r : 32 * r + 17, kbase : kbase + 128],
                        rhs=qk[u][32 * r : 32 * r + 17, qs : qs + CHUNK],
                        start=True,
                        stop=True,
                        tile_position=(32 * r, 0),
                    )
                pt = ptp.tile([C, CHUNK * len(tlist)], bf16, name="pt_sb", tag="pt")
                if eng_act[idx]:
                    nc.scalar.activation(pt[:], sc[:], Exp, bias=0.0, scale=0.25)
                else:
                    nc.vector.tensor_scalar(
                        pt[:].bitcast(i16), sc[:], A_SCH, B_SCH, op0=Mult, op1=Add
                    )
                pt_of_item[idx] = pt

            def emit_chunk_epilogue(c):
                acc = acc_of_chunk.pop(c)
                nc.vector.tensor_copy(dstage[0:96, :], acc[0:96, :])
                for u in range(3):
                    b = 32 * u
                    nc.sync.dma_start(
                        denb[b : b + 32, :], dstage[b + 16 : b + 17, :]
                    )
                nc.vector.reciprocal(recb[0:96, :], denb[0:96, :])
                for u in range(3):
                    b = 32 * u
                    nc.sync.dma_start(normsb[b : b + 1, :], recb[b : b + 32, :])
                    for w in (1, 2, 4, 8):
                        nc.sync.dma_start(
                            normsb[b + w : b + 2 * w, :], normsb[b : b + w, :]
                        )
                for u in range(3):
                    b = 32 * u
                    nc.vector.tensor_mul(
                        scaled[b : b + 16, :],
                        acc[b : b + 16, :],
                        normsb[b : b + 16, :],
                    )

            def emit_attnv(idx):
                c, g, u = items[idx]
                tlist = groups[g]
                if c not in acc_of_chunk:
                    acc_of_chunk[c] = accp.tile([C, CHUNK], f32, name="acc_ps", tag="acc")
                acc = acc_of_chunk[c]
                pt = pt_of_item.pop(idx)
                for i, t in enumerate(tlist):
                    nc.tensor.matmul(
                        acc[32 * u : 32 * u + 17, :],
                        lhsT=vT[:, 51 * t + 17 * u : 51 * t + 17 * u + 17],
                        rhs=pt[:, i * CHUNK : (i + 1) * CHUNK],
                        start=(g == 0),
                        stop=(g == NGRP - 1),
                        tile_position=(0, 32 * u),
                    )

            def emit_proj(c):
                cs, ce = c * CHUNK, (c + 1) * CHUNK
                pj = projp.tile([C, CHUNK], f32, name="pj_ps", tag="proj")
                nc.tensor.matmul(
                    pj[:], lhsT=wo[:], rhs=scaled[:], start=True, stop=True
                )
                osb = osbp.tile([C, CHUNK], f32, name="osb_sb", tag="osb")
                nc.vector.tensor_copy(osb[:], pj[:])
                nc.sync.dma_start(y_d[:, cs:ce], osb[:])

            # seeds: chunk-0 projections + first vT batch
            for u in range(3):
                emit_qk(u, 0)
            emit_vt(0)

            di = 0
            for idx in range(n_items + LAG + EPI_DELAY + 2):
                while pending_proj and pending_proj[0][0] <= idx:
                    emit_proj(pending_proj.pop(0)[1])
                while di < len(drip) and drip[di][0] <= idx:
                    _, kind, arg = drip[di]
                    di += 1
                    if kind == "qk":
                        emit_qk(*arg)
                    else:
                        emit_vt(arg)
                if idx < n_items:
                    emit_scores(idx)
                av = idx - LAG
                if 0 <= av < n_items:
                    emit_attnv(av)
                    if (av + 1) % per_chunk == 0:
                        cdone = av // per_chunk
                        emit_chunk_epilogue(cdone)
                        pending_proj.append((idx + EPI_DELAY, cdone))
            while pending_proj:
                emit_proj(pending_proj.pop(0)[1])

    if split_waits:
        _split_multi_waits(nc)
    return nc


def host_prep(inputs, cfg=_FULL):
    """Slice/pack the full problem inputs into per-core input maps."""
    CHUNK = cfg["CHUNK"]
    N = cfg["MT"] * 128
    bf = _bf16np()

    x = np.asarray(inputs["x"], dtype=np.float32)
    B = x.shape[0]
    assert B == 1
    xf = np.ascontiguousarray(x.reshape(C, -1))[:, :N]

    gamma0 = float(np.asarray(inputs["gamma"]).reshape(-1)[0])
    branches = [
        (
            np.asarray(inputs[f"w_qkv_{nm}"], dtype=np.float32),
            np.asarray(inputs[f"b_qkv_{nm}"], dtype=np.float32),
            np.asarray(inputs[f"w_out_{nm}"], dtype=np.float32),
            np.asarray(inputs[f"b_out_{nm}"], dtype=np.float32),
        )
        for nm in ("d", "h", "w")
    ]

    beff_total = np.zeros(C, dtype=np.float64)
    for wqkv, bqkv, wout, bout in branches:
        bv = bqkv[2 * C : 3 * C]
        beff_total += gamma0 * (wout.astype(np.float64) @ bv + bout)
    beff_core = (beff_total / NCORES).astype(np.float32)

    addv = np.zeros((C, 1), dtype=np.float32)
    for r in range(4):
        addv[32 * r + 16, 0] = 1.0
    onesr = np.zeros((C, CHUNK), dtype=np.float32)
    onesr[96, :] = 1.0

    in_maps = []
    for h in range(NCORES):
        m = {
            "x": xf.astype(bf),
            "addv": addv,
            "onesr": onesr,
        }
        wv3 = np.zeros((C, 52), dtype=np.float32)
        wo_stacked = np.zeros((C, 128), dtype=np.float32)
        wo_stacked[96, :] = beff_core
        for u, (wqkv, bqkv, wout, bout) in enumerate(branches):
            wq = wqkv[h * DH : (h + 1) * DH, :]  # (16, 128)
            wk = wqkv[C + h * DH : C + (h + 1) * DH, :]
            wvu = wqkv[2 * C + h * DH : 2 * C + (h + 1) * DH, :]
            bqu = bqkv[h * DH : (h + 1) * DH]
            rvec = wk.T @ bqu  # (128,)

            lqm = np.zeros((C, 128), dtype=np.float32)
            lkm = np.zeros((C, 128), dtype=np.float32)
            for r in range(4):
                lqm[:, 32 * r : 32 * r + 16] = wq.T
                lkm[:, 32 * r : 32 * r + 16] = wk.T
                lkm[:, 32 * r + 16] = rvec
            m[f"lq{u}"] = lqm.astype(bf)
            m[f"lk{u}"] = lkm.astype(bf)

            wv3[:, u * 17 : u * 17 + 16] = wvu.T  # col 16 stays 0 (ones memset)
            wo_stacked[32 * u : 32 * u + 16, :] = (
                gamma0 * wout[:, h * DH : (h + 1) * DH].T
            )
        m["wv3"] = wv3.astype(bf)
        m["wo"] = wo_stacked
        in_maps.append(m)
    return in_maps


def gather(results, inputs, cfg=_FULL):
    x = np.asarray(inputs["x"], dtype=np.float32)
    N = cfg["MT"] * 128
    acc = np.zeros((C, N), dtype=np.float32)
    for r in results:
        acc += r["y"]
    out = acc + x.reshape(C, -1)[:, :N]
    return out.reshape(x.shape).astype(np.float32)


def kernel(**inputs) -> np.ndarray:
    from concourse.bass_utils import run_bass_kernel_spmd

    if "nc" not in _CACHE:
        _CACHE["nc"] = build_nc(_FULL)
    nc = _CACHE["nc"]
    in_maps = host_prep(inputs, _FULL)
    res = run_bass_kernel_spmd(nc, in_maps, list(range(NCORES)))
    return gather(res.results, inputs, _FULL)
